# revision 39
# baseline (speedup 1.0000x reference)
"""YOLO-style loss kernel for Trainium2, 8-core data-parallel.

Strategy: shard the 16384 batch across 8 cores (2048 each = 100352 grid
cells). Each core streams its [cells, 30] fp32 pred/target arrays through
SBUF in 4 wide tiles. Per tile, all five loss terms are reduced to masked
values in one bf16 scratch strip [128, 196, 32] which a single scalar-engine
ACTIVATE(Square, accum_out=...) reduces per tile; term weights (5, 0.5) are
pre-folded into the masks, so each tile needs exactly one accumulate.

The IoU block avoids corner arithmetic: per-axis interval overlap is
  ow = (wp + wt)/2 - max(|cxp - cxt|, |wp - wt|/2),   cx = x/7
which needs only pairwise sums/diffs (i=pred box, j=target box), an Abs on
the scalar engine, one max and one sub on DVE (both 2x bf16). Work is split
across DVE / GpSimd(Pool) / Act so each engine stays under the ~67us DMA
roofline (24 MB/core at ~360 GB/s).

Per-cell math (channels [x0,y0,w0,h0,c0, x1,y1,w1,h1,c1, 20 class]):
  obj  = t4 > 0, noo = t4 == 0
  iou(i,j) from inter = relu(owx)*relu(owy), union = areap_i+areat_j-inter
  g_j = iou(1,j) > iou(0,j), m_j = max_i iou(i,j)
  conf targets ct0 = m1 + g1*(m0-m1), ct1 = m0 - g1*(m0-m1)  (masked wrong
  cases are killed by resp)
  resp_0 = obj*(1-min(g0,g1)), resp_1 = obj*max(g0,g1)
  strip lanes: [0:20] obj*dcls | [20:22] resp*(pc-ct) | [22:24] noo*sqrt(.5)*d49
   | [24:28] resp*sqrt(5)*3.5*(|dxy|*2/7) | [28:32] resp*sqrt(5)*(sqrt(wh+e) diff)
"""

import math

import numpy as np
import concourse.bass as bass
import concourse.tile as tile
from concourse import mybir
from concourse.bass_utils import run_bass_kernel_spmd

F32 = mybir.dt.float32
BF16 = mybir.dt.bfloat16
Alu = mybir.AluOpType
Act = mybir.ActivationFunctionType

# problem constants (hardcoded per harness contract)
BATCH = 16384
S = 7
D = 30
N_CORES = 8
B_PER = BATCH // N_CORES            # 2048
K_CORE = B_PER * S * S              # 100352 cells/core
P = 128
CELLS_PER_PART = K_CORE // P        # 784
NT = 4                              # tiles per core
CPP = CELLS_PER_PART // NT          # 196 cells per partition per tile
EPS = 1e-6
SQRT5 = math.sqrt(5.0)
SQRT_HALF = math.sqrt(0.5)
CLS_POOL = 12                       # class channels masked on Pool (rest DVE)
NGROUPS = 1
RECIP = "hw"                        # "hw" (InstReciprocal) | "newton" (seed+1NR)
U32 = mybir.dt.uint32
RECIP_MAGIC = 0x7EF311C3


def split_sync_waits(nc, max_attached=1):
    """This container's walrus build rejects >1 semaphore wait attached to an
    instruction. Hoist the extras into standalone EventSemaphore wait
    instructions (what raw-bass wait_ge emits), which it accepts."""
    n = 0
    for func in nc.m.functions:
        for bb in func.blocks:
            insts = list(bb.instructions)
            out = []
            changed = False
            for inst in insts:
                si = inst.sync_info
                if si is not None and len(si.on_wait) > max_attached:
                    waits = list(si.on_wait)
                    keep, hoist = waits[:max_attached], waits[max_attached:]
                    for k, w in enumerate(hoist):
                        wi = mybir.InstEventSemaphore(
                            name=f"{inst.name}-hw{k}", ins=[], outs=[]
                        )
                        wi.engine = inst.engine
                        wi.sync_info = mybir.SyncInfo(on_wait=[w], on_update=[])
                        nc.register_instruction(wi, overwrite=True)
                        out.append(wi)
                        n += 1
                    inst.sync_info = mybir.SyncInfo(
                        on_wait=keep, on_update=list(si.on_update)
                    )
                    changed = True
                out.append(inst)
            if changed:
                while len(bb.instructions):
                    bb.instructions.pop()
                for i in out:
                    bb.instructions.append(i)
    return n


def mkap(t_ap, off, dims):
    """AP into a [P, F] tile/view: keep partition dim, custom free dims.
    dims = list of [stride_elems, count]."""
    return bass.AP(tensor=t_ap.tensor, offset=t_ap.offset + off,
                   ap=[list(t_ap.ap[0])] + [list(d) for d in dims])


def ch(t, c0, dims):
    """Box-channel view of an io tile t ([P, CPP*30]): cell-major, channel c0,
    extra dims appended after the cell dim."""
    return mkap(t[:], c0, [[D, CPP]] + dims)


def bc(ap, reps):
    """Replace a trailing singleton dim with a zero-stride broadcast dim."""
    new = [list(d) for d in ap.ap]
    assert new[-1][1] == 1, new
    new[-1] = [0, reps]
    return bass.AP(tensor=ap.tensor, offset=ap.offset, ap=new)


def build_kernel(repeat=1, timing=False):
    nc = bass.Bass("TRN2")
    kind = "Internal" if timing else "ExternalInput"
    pred = nc.dram_tensor("pred", [K_CORE, D], F32, kind=kind)
    targ = nc.dram_tensor("targ", [K_CORE, D], F32, kind=kind)
    NTR = NT * repeat
    out = nc.dram_tensor("out", [P, NTR], F32, kind="ExternalOutput")

    pred_v = pred.ap().rearrange("(n p c) d -> n p (c d)", n=NT, p=P, c=CPP)
    targ_v = targ.ap().rearrange("(n p c) d -> n p (c d)", n=NT, p=P, c=CPP)

    with tile.TileContext(nc) as tc:
        with (
            tc.tile_pool(name="io", bufs=2) as io,
            tc.tile_pool(name="strip", bufs=2) as sp,
            tc.tile_pool(name="mid", bufs=2) as mid,
            tc.tile_pool(name="mid1", bufs=2) as mid1,
            tc.tile_pool(name="accp", bufs=1) as accp,
        ):
            acc = accp.tile([P, NTR], F32)
            eps_t = accp.tile([P, 1], F32)
            zero_t = accp.tile([P, 1], F32)
            nc.vector.memset(eps_t[:], EPS)
            nc.vector.memset(zero_t[:], 0.0)

            for rit in range(NTR):
                it = rit % NT
                pt = io.tile([P, CPP * D], F32, tag="pt")
                tt = io.tile([P, CPP * D], F32, tag="tt")
                nc.sync.dma_start(out=pt[:], in_=pred_v[it])
                nc.sync.dma_start(out=tt[:], in_=targ_v[it])

                strip = sp.tile([P, CPP, 32], BF16, tag="strip")

                # ---- pairwise sums and diffs [P, C, 2d, 2j, 2i] ----
                # pred channel dep: i only; targ: j only. d: w/h (or x/y).
                sxy = mid.tile([P, CPP, 8], BF16, tag="sxy")
                aaw = mid1.tile([P, CPP, 16], BF16, tag="aaw")
                # ISA allows max 3 free dims: one instr per d (x/y, w/h)
                for dd in range(2):
                    s_out = mkap(sxy[:], 4 * dd, [[8, CPP], [2, 2], [1, 2]])
                    nc.vector.tensor_tensor(
                        out=s_out,
                        in0=ch(pt, 2 + dd, [[0, 2], [5, 2]]),
                        in1=ch(tt, 2 + dd, [[5, 2], [0, 2]]), op=Alu.add)
                    a_out = mkap(aaw[:], 4 * dd, [[16, CPP], [2, 2], [1, 2]])
                    nc.vector.tensor_tensor(
                        out=a_out,
                        in0=ch(pt, 0 + dd, [[0, 2], [5, 2]]),
                        in1=ch(tt, 0 + dd, [[5, 2], [0, 2]]), op=Alu.subtract)
                    w_out = mkap(aaw[:], 8 + 4 * dd, [[16, CPP], [2, 2], [1, 2]])
                    nc.vector.tensor_tensor(
                        out=w_out,
                        in0=ch(pt, 2 + dd, [[0, 2], [5, 2]]),
                        in1=ch(tt, 2 + dd, [[5, 2], [0, 2]]), op=Alu.subtract)

                # |a|*2/7 and |dw| in place (Act)
                aa = mkap(aaw[:], 0, [[16, CPP], [1, 8]])
                ww = mkap(aaw[:], 8, [[16, CPP], [1, 8]])
                nc.scalar.activation(out=aa, in_=aa, func=Act.Abs,
                                     bias=zero_t[:], scale=2.0 / S)
                nc.scalar.activation(out=ww, in_=ww, func=Act.Abs,
                                     bias=zero_t[:], scale=1.0)

                # hm = max(|a|2/7, |dw|); q = s - hm (both 2x bf16)
                hm = mid.tile([P, CPP, 8], BF16, tag="hm")
                nc.vector.tensor_tensor(out=hm[:], in0=aa, in1=ww, op=Alu.max)
                nc.vector.tensor_tensor(out=sxy[:], in0=sxy[:], in1=hm[:],
                                        op=Alu.subtract)
                # oc = relu(0.5*q) (Act)
                oc = mid.tile([P, CPP, 8], BF16, tag="oc")
                nc.scalar.activation(out=oc[:], in_=sxy[:], func=Act.Relu,
                                     bias=zero_t[:], scale=0.5)

                # inter[j,i] = ow_x * ow_y (2x)
                inter = mid.tile([P, CPP, 4], BF16, tag="inter")
                o_x = mkap(oc[:], 0, [[8, CPP], [2, 2], [1, 2]])
                o_y = mkap(oc[:], 4, [[8, CPP], [2, 2], [1, 2]])
                nc.vector.tensor_tensor(out=inter[:], in0=o_x, in1=o_y,
                                        op=Alu.mult)

                # areas + pairwise union (Pool), then 1/union (DVE)
                areap = mid.tile([P, CPP, 2], F32, tag="areap")
                areat = mid.tile([P, CPP, 2], F32, tag="areat")
                nc.gpsimd.tensor_tensor(out=areap[:], in0=ch(pt, 2, [[5, 2]]),
                                        in1=ch(pt, 3, [[5, 2]]), op=Alu.mult)
                nc.gpsimd.tensor_tensor(out=areat[:], in0=ch(tt, 2, [[5, 2]]),
                                        in1=ch(tt, 3, [[5, 2]]), op=Alu.mult)
                uni = mid.tile([P, CPP, 2, 2], F32, tag="uni")
                ap_b = mkap(areap[:], 0, [[2, CPP], [0, 2], [1, 2]])
                at_b = mkap(areat[:], 0, [[2, CPP], [1, 2], [0, 2]])
                nc.gpsimd.tensor_tensor(out=uni[:], in0=ap_b, in1=at_b,
                                        op=Alu.add)
                uni4 = uni[:].rearrange("p c a b -> p (c a b)")
                inter4 = inter[:].rearrange("p c k -> p (c k)")
                nc.vector.tensor_tensor(out=uni4, in0=uni4, in1=inter4,
                                        op=Alu.subtract)
                run = mid.tile([P, CPP, 4], F32, tag="run")
                run4 = run[:].rearrange("p c k -> p (c k)")
                if RECIP == "hw":
                    nc.vector.reciprocal(out=run4, in_=uni4)
                else:
                    # seed via magic-constant exponent flip, then 1 Newton step
                    # magic - u == (u ^ 0xFFFFFFFF) + (magic+1)  (mod 2^32)
                    nc.vector.tensor_scalar(
                        out=run4.bitcast(U32), in0=uni4.bitcast(U32),
                        scalar1=0xFFFFFFFF, scalar2=None, op0=Alu.bitwise_xor)
                    nc.vector.tensor_scalar(
                        out=run4.bitcast(U32), in0=run4.bitcast(U32),
                        scalar1=RECIP_MAGIC + 1, scalar2=None, op0=Alu.add)
                    rtmp = mid.tile([P, CPP * 4], F32, tag="rtmp")
                    nc.vector.tensor_tensor(out=rtmp[:], in0=uni4, in1=run4,
                                            op=Alu.mult)
                    nc.vector.tensor_scalar(out=rtmp[:], in0=rtmp[:],
                                            scalar1=-1.0, scalar2=2.0,
                                            op0=Alu.mult, op1=Alu.add)
                    nc.vector.tensor_tensor(out=run4, in0=run4, in1=rtmp[:],
                                            op=Alu.mult)

                # iou, transposed to [i][j] so m/g run 2x over packed j
                iou = mid.tile([P, CPP, 2, 2], BF16, tag="iou")  # [i][j]
                iou_t = mkap(iou[:], 0, [[4, CPP], [1, 2], [2, 2]])  # [c][j][i]
                nc.vector.tensor_tensor(out=iou_t, in0=inter[:], in1=run[:],
                                        op=Alu.mult)
                m = mid.tile([P, CPP, 2], BF16, tag="m")
                g = mid.tile([P, CPP, 2], BF16, tag="g")
                iou_i0 = mkap(iou[:], 0, [[4, CPP], [1, 2]])
                iou_i1 = mkap(iou[:], 2, [[4, CPP], [1, 2]])
                nc.vector.tensor_tensor(out=m[:], in0=iou_i0, in1=iou_i1,
                                        op=Alu.max)
                nc.vector.tensor_tensor(out=g[:], in0=iou_i1, in1=iou_i0,
                                        op=Alu.is_gt)

                # conf targets + responsibility masks (Pool)
                m0, m1 = m[:, :, 0:1], m[:, :, 1:2]
                g0, g1 = g[:, :, 0:1], g[:, :, 1:2]
                dm = mid.tile([P, CPP, 1], BF16, tag="dm")
                gdm = mid.tile([P, CPP, 1], BF16, tag="gdm")
                ct = mid.tile([P, CPP, 2], BF16, tag="ct")
                nc.gpsimd.tensor_tensor(out=dm[:], in0=m0, in1=m1, op=Alu.subtract)
                nc.gpsimd.tensor_tensor(out=gdm[:], in0=g1, in1=dm[:], op=Alu.mult)
                nc.gpsimd.tensor_tensor(out=ct[:, :, 0:1], in0=m1, in1=gdm[:], op=Alu.add)
                nc.gpsimd.tensor_tensor(out=ct[:, :, 1:2], in0=m0, in1=gdm[:], op=Alu.subtract)

                obj = mid.tile([P, CPP, 1], BF16, tag="obj")
                noo = mid.tile([P, CPP, 1], BF16, tag="noo")
                t4 = ch(tt, 4, [[1, 1]])
                nc.gpsimd.tensor_scalar(out=obj[:], in0=t4, scalar1=0.0,
                                        scalar2=None, op0=Alu.is_gt)
                nc.gpsimd.tensor_scalar(out=noo[:], in0=t4, scalar1=0.0,
                                        scalar2=None, op0=Alu.is_le)
                nc.gpsimd.tensor_scalar(out=noo[:], in0=noo[:], scalar1=SQRT_HALF,
                                        scalar2=None, op0=Alu.mult)
                gmin = mid.tile([P, CPP, 1], BF16, tag="gmin")
                rr = mid.tile([P, CPP, 2], BF16, tag="rr")
                # g binary: min(g0,g1) == g0*g1 (Pool tt: add/sub/mult only)
                nc.gpsimd.tensor_tensor(out=gmin[:], in0=g0, in1=g1, op=Alu.mult)
                nc.gpsimd.tensor_scalar(out=rr[:, :, 0:1], in0=gmin[:],
                                        scalar1=-1.0, scalar2=1.0,
                                        op0=Alu.mult, op1=Alu.add)
                nc.vector.tensor_tensor(out=rr[:, :, 1:2], in0=g0, in1=g1, op=Alu.max)
                rm = mid.tile([P, CPP, 2], BF16, tag="rm")
                rm5 = mid.tile([P, CPP, 2], BF16, tag="rm5")
                rm5x = mid.tile([P, CPP, 2], BF16, tag="rm5x")
                nc.gpsimd.tensor_tensor(out=rm[:], in0=rr[:], in1=bc(obj[:], 2),
                                        op=Alu.mult)
                nc.gpsimd.tensor_scalar(out=rm5[:], in0=rm[:], scalar1=SQRT5,
                                        scalar2=None, op0=Alu.mult)
                nc.gpsimd.tensor_scalar(out=rm5x[:], in0=rm[:], scalar1=SQRT5 * S / 2.0,
                                        scalar2=None, op0=Alu.mult)

                # contain: (pc - ct) * rm -> strip[20:22]
                st_e = mkap(strip[:], 20, [[32, CPP], [1, 2]])
                nc.vector.tensor_tensor(out=st_e, in0=ch(pt, 4, [[5, 2]]),
                                        in1=ct[:], op=Alu.subtract)
                nc.vector.tensor_tensor(out=st_e, in0=st_e, in1=rm[:], op=Alu.mult)

                # noobj: (p49 - t49) * noo*sqrt(.5) -> strip[22:24] (Pool)
                st_n = mkap(strip[:], 22, [[32, CPP], [1, 2]])
                nc.gpsimd.tensor_tensor(out=st_n, in0=ch(pt, 4, [[5, 2]]),
                                        in1=ch(tt, 4, [[5, 2]]), op=Alu.subtract)
                nc.gpsimd.tensor_tensor(out=st_n, in0=st_n,
                                        in1=bc(noo[:], 2), op=Alu.mult)

                # loc xy: diag(|a|*2/7) * rm5x -> strip[24:28]  [d][b]
                adiag = mkap(aaw[:], 0, [[16, CPP], [4, 2], [3, 2]])
                st_xy = mkap(strip[:], 24, [[32, CPP], [2, 2], [1, 2]])
                rm5x_b = mkap(rm5x[:], 0, [[2, CPP], [0, 2], [1, 2]])
                nc.vector.tensor_tensor(out=st_xy, in0=adiag, in1=rm5x_b,
                                        op=Alu.mult)

                # loc wh: (sqrt(p_wh+eps)-sqrt(t_wh+eps)) * rm5 -> strip[28:32]
                sqp = mid.tile([P, CPP, 2, 2], BF16, tag="sqp")  # [b][d]
                sqt = mid.tile([P, CPP, 2, 2], BF16, tag="sqt")
                nc.scalar.activation(out=sqp[:], in_=ch(pt, 2, [[5, 2], [1, 2]]),
                                     func=Act.Sqrt, bias=eps_t[:], scale=1.0)
                nc.scalar.activation(out=sqt[:], in_=ch(tt, 2, [[5, 2], [1, 2]]),
                                     func=Act.Sqrt, bias=eps_t[:], scale=1.0)
                st_wh = mkap(strip[:], 28, [[32, CPP], [1, 4]])
                nc.vector.tensor_tensor(
                    out=st_wh, in0=sqp[:].rearrange("p c a b -> p c (a b)"),
                    in1=sqt[:].rearrange("p c a b -> p c (a b)"), op=Alu.subtract)
                st_wh2 = mkap(strip[:], 28, [[32, CPP], [2, 2], [1, 2]])
                rm5_b = mkap(rm5[:], 0, [[2, CPP], [1, 2], [0, 2]])
                nc.vector.tensor_tensor(out=st_wh2, in0=st_wh2, in1=rm5_b,
                                        op=Alu.mult)

                # class: dcls -> strip[0:20] (DVE), obj-mask split Pool/DVE
                st_c = mkap(strip[:], 0, [[32, CPP], [1, 20]])
                nc.vector.tensor_tensor(out=st_c, in0=ch(pt, 10, [[1, 20]]),
                                        in1=ch(tt, 10, [[1, 20]]), op=Alu.subtract)
                kp = CLS_POOL
                st_cp = mkap(strip[:], 0, [[32, CPP], [1, kp]])
                nc.gpsimd.tensor_tensor(out=st_cp, in0=st_cp,
                                        in1=bc(obj[:], kp), op=Alu.mult)
                st_cv = mkap(strip[:], kp, [[32, CPP], [1, 20 - kp]])
                nc.vector.tensor_tensor(out=st_cv, in0=st_cv,
                                        in1=bc(obj[:], 20 - kp), op=Alu.mult)

                # single fused square+accumulate for the whole strip
                s_flat = strip[:].rearrange("p c w -> p (c w)")
                nc.scalar.activation(out=s_flat, in_=s_flat, func=Act.Square,
                                     scale=1.0, accum_out=acc[:, rit:rit + 1])

            nc.sync.dma_start(out=out[:], in_=acc[:])

    split_sync_waits(nc)
    return nc


_NC_CACHE = None


def kernel(pred_tensor: np.ndarray, target_tensor: np.ndarray) -> np.ndarray:
    global _NC_CACHE
    if _NC_CACHE is None:
        _NC_CACHE = build_kernel()
    nc = _NC_CACHE

    p = np.ascontiguousarray(pred_tensor, dtype=np.float32).reshape(N_CORES, K_CORE, D)
    t = np.ascontiguousarray(target_tensor, dtype=np.float32).reshape(N_CORES, K_CORE, D)
    in_maps = [{"pred": p[i], "targ": t[i]} for i in range(N_CORES)]
    res = run_bass_kernel_spmd(nc, in_maps, core_ids=list(range(N_CORES)))
    total = 0.0
    for i in range(N_CORES):
        total += res.results[i]["out"].astype(np.float64).sum()
    return np.float32(total / BATCH)
